# revision 16
# baseline (speedup 1.0000x reference)
"""Multi-head attention (dense_transformer) on 8 TRN2 NeuronCores.

Sharding: 2-way data parallel over batch x 4-way tensor parallel over heads.
Core c handles batch b=c//4 and heads {4g..4g+3} where g=c%4 (4 heads, 256
channels per core; channels of head h are qw columns {hd*16+h}).

Architecture (v2, "transposed scores"):
  phase 1: Q^T/K^T projections in [ch, s] layout via 3-term bf16 splits
           (pseudo-fp32, needed because softmax here is a near-argmax: score
           std ~256, so score errors flip the max). V is projected DIRECTLY
           into [s, ch] layout with single-pass f32r matmuls (V error is
           linear in the output -> 11-bit mantissa is plenty), with a ones
           column appended per head so AV also produces the softmax sums.
  phase 2: scores computed TRANSPOSED: scT[k,q] = K^T-chunk (stationary) x
           Q^T (moving), 3-term bf16. Per 512-wide q block: drain chunks to
           SBUF, running column-max on gpsimd (Pool engine, otherwise idle),
           one partition_all_reduce(max) -> bias replicated on all
           partitions, DVE subtract + ACT exp -> expT bf16, then
           AV = Vhat-chunk (stationary [128,65]) x expT (moving) accumulates
           O^T[ch,q] AND l[q] in PSUM with no transposes at all.
           Normalize = reciprocal of l + gpsimd partition_broadcast + the
           PSUM drain is a tensor_tensor multiply.
  phase 3: AllGather O^T across the 4 cores of the batch -> out-proj column
           slice (out^T = ow_perm^T @ merged^T, f32r) -> DMA out.

vs v1: no PE transposes (was 576 matmuls / ~97us), V projection 3x cheaper,
no separate normalize pass, no exp accum; PE stream is denser so it holds
the 2.4GHz p-state instead of 1.2GHz.
"""
import sys

sys.path.insert(0, "/opt/trn_rl_repo")

import numpy as np

import concourse.bass as bass
import concourse.mybir as mybir
import concourse.tile as tile
from concourse import bacc
from concourse import bass_isa
from concourse.bass_utils import run_bass_kernel_spmd

# ---- problem constants (hardcoded per harness contract) ----
B, S, D, HEADS = 2, 2048, 1024, 16
N_CORES = 8
GROUPS = 4                 # head-groups == cores per batch
HPC = HEADS // GROUPS      # heads per core (4)
HD = D // HEADS            # 64
CPC = HPC * HD             # channels per core (256)
P = 128
NCC = CPC // P             # col chunks per core (2)
DCH = D // P               # contraction chunks (8)
QB = 512                   # q block width (1 PSUM bank of f32)

f32 = mybir.dt.float32
f32r = mybir.dt.float32r
bf16 = mybir.dt.bfloat16

AX = mybir.AxisListType
EXP = mybir.ActivationFunctionType.Exp
MAXOP = mybir.AluOpType.max
SUB = mybir.AluOpType.subtract
MULT = mybir.AluOpType.mult
COPYF = mybir.ActivationFunctionType.Copy

DEFAULT_CFG = dict(s=S)


def make_maskT(nc, maskT, mask_val=-1e10):
    """maskT[k, q] = 0 if q >= k else mask_val (transposed causal)."""
    sq = maskT.shape[0]
    nc.gpsimd.memset(maskT, mask_val)
    nc.gpsimd.affine_select(
        out=maskT,
        in_=maskT,
        compare_op=mybir.AluOpType.is_gt,
        fill=0.0,
        base=0,
        # keep mask_val where (k - q) > 0, else fill 0
        pattern=[[-1, sq]],
        channel_multiplier=1,
    )


def build_nc(s=S, dbg=False):
    assert s % QB == 0
    NQB = s // QB            # 512-wide q blocks
    NKC = s // P             # 128-wide k chunks
    KPB = QB // P            # k chunks per q block on the diagonal (4)
    VW = 65                  # V channels per (hp,h2) incl the ones column
    NH2 = NCC * 2            # head slots per core (4)

    nc = bacc.Bacc("TRN2", target_bir_lowering=False, debug=False,
                   num_devices=N_CORES)
    xT = nc.dram_tensor("xT", [D, s], f32, kind="ExternalInput").ap()
    wq = nc.dram_tensor("wq", [D, CPC], f32, kind="ExternalInput").ap()
    wk = nc.dram_tensor("wk", [D, CPC], f32, kind="ExternalInput").ap()
    wv = nc.dram_tensor("wv", [D, CPC], f32r, kind="ExternalInput").ap()
    wo = nc.dram_tensor("wo", [D, CPC], f32, kind="ExternalInput").ap()
    outT = nc.dram_tensor("outT", [NCC, P, s], f32, kind="ExternalOutput").ap()

    with tile.TileContext(nc) as tc:
        with (
            tc.tile_pool(name="cpool", bufs=1) as cpool,
            tc.tile_pool(name="wpool", bufs=1) as wpool,
            tc.tile_pool(name="big", bufs=1) as big,
            tc.tile_pool(name="stat", bufs=2) as stat,
            tc.tile_pool(name="ms", bufs=6) as ms,
            tc.tile_pool(name="op", bufs=2) as op,
            tc.tile_pool(name="dram", bufs=1, space="DRAM") as dpool,
        ):
            NQB_ = s // QB
            ag_in = {qb: dpool.tile([P, NCC, QB], bf16, tag=f"agi{qb}",
                                    name=f"agi{qb}")
                     for qb in range(NQB_ - 1)}
            ag_out = {qb: dpool.tile([GROUPS, P, NCC, QB], bf16,
                                     tag=f"ago{qb}", name=f"ago{qb}")
                      for qb in range(NQB_ - 1)}
            ag_in_h = {(NQB_ - 1, hp): dpool.tile([P, QB], bf16,
                                                  tag=f"agih{hp}",
                                                  name=f"agih{hp}")
                       for hp in range(NCC)}
            ag_out_h = {(NQB_ - 1, hp): dpool.tile([GROUPS, P, QB], bf16,
                                                   tag=f"agoh{hp}",
                                                   name=f"agoh{hp}")
                        for hp in range(NCC)}

            maskT = cpool.tile([P, P], f32, tag="maskT")
            make_maskT(nc, maskT[:])

            woh = wpool.tile([P, DCH, CPC], bf16, tag="woh")
            wol = wpool.tile([P, DCH, CPC], bf16, tag="wol")
            wv_sb = wpool.tile([P, DCH, CPC], f32r, tag="wv")
            for di in range(DCH):
                nc.sync.dma_start(wv_sb[:, di, :], wv[di * P:(di + 1) * P, :])
            wsplit = {}
            for nm in ("q", "k"):
                wh = wpool.tile([P, DCH, CPC], bf16, tag=f"w{nm}h", name=f"w{nm}h")
                wl = wpool.tile([P, DCH, CPC], bf16, tag=f"w{nm}l", name=f"w{nm}l")
                wsplit[nm] = [wh, wl]
            with tc.tile_pool(name="wload", bufs=2) as wload:
                for nm, wdr in (("q", wq), ("k", wk), ("o", wo)):
                    wf = wload.tile([P, DCH, CPC], f32, tag="wf", name="wf")
                    for di in range(DCH):
                        nc.sync.dma_start(wf[:, di, :],
                                          wdr[di * P:(di + 1) * P, :])
                    wh, wl = ((woh, wol) if nm == "o" else wsplit[nm])
                    nc.vector.tensor_copy(wh[:], wf[:])
                    nc.vector.tensor_tensor(wl[:], wf[:], wh[:], SUB)

            QTh = big.tile([P, NCC, s], bf16, tag="QTh")
            QTl = big.tile([P, NCC, s], bf16, tag="QTl")
            KTh = big.tile([P, NCC, s], bf16, tag="KTh")
            KTl = big.tile([P, NCC, s], bf16, tag="KTl")
            # Vhat[k, :]: 4 groups of 65 cols: 64 V channels + a ones col
            Vsb = big.tile([P, NKC, NH2 * VW], bf16, tag="Vsb")
            OT = big.tile([P, NCC, s], bf16, tag="OT")
            stage = big.tile([P, NKC, QB], f32, tag="stage")
            expT = big.tile([P, NKC, QB], bf16, tag="expT")

            for g in range(NH2):
                nc.gpsimd.memset(Vsb[:, :, g * VW + 64], 1.0)

            # ---------------- phase 1: projections ----------------
            with (
                tc.tile_pool(name="psp", bufs=1, space="PSUM") as psp,
                tc.tile_pool(name="psv", bufs=1, space="PSUM") as psv,
                tc.tile_pool(name="xs", bufs=5) as xs,
            ):
                for qb in range(NQB):
                    accs = {}
                    for nm in ("q", "k"):
                        for cc in range(NCC):
                            accs[nm, cc] = psp.tile([P, QB], f32,
                                                    tag=f"pp{nm}{cc}",
                                                    name=f"pp{nm}{cc}")
                    vacc = [psv.tile([P, CPC], f32, tag=f"pv{r}", name=f"pv{r}")
                            for r in range(KPB)]
                    for di in range(DCH):
                        xt = xs.tile([P, QB], f32, tag="xt", name="xt")
                        nc.sync.dma_start(
                            xt[:], xT[di * P:(di + 1) * P, qb * QB:(qb + 1) * QB])
                        xth = xs.tile([P, QB], bf16, tag="xth", name="xth")
                        xtl = xs.tile([P, QB], bf16, tag="xtl", name="xtl")
                        nc.vector.tensor_copy(xth[:], xt[:])
                        nc.vector.tensor_tensor(xtl[:], xt[:], xth[:], SUB)
                        xtr = xs.tile([P, QB], f32r, tag="xtr", name="xtr")
                        nc.any.tensor_copy(xtr[:], xt[:])
                        for nm in ("q", "k"):
                            wh, wl = wsplit[nm]
                            for cc in range(NCC):
                                csl = slice(cc * P, (cc + 1) * P)
                                terms = [(wh, xth), (wh, xtl), (wl, xth)]
                                for ti, (wt, xtt) in enumerate(terms):
                                    nc.tensor.matmul(
                                        accs[nm, cc][:], wt[:, di, csl], xtt[:],
                                        start=(di == 0 and ti == 0),
                                        stop=(di == DCH - 1 and ti == len(terms) - 1))
                        for r in range(KPB):
                            nc.tensor.matmul(
                                vacc[r][:], xtr[:, r * P:(r + 1) * P],
                                wv_sb[:, di, :],
                                start=(di == 0), stop=(di == DCH - 1))
                    sl = slice(qb * QB, (qb + 1) * QB)
                    for cc in range(NCC):
                        for hi_t, lo_t, ps in ((QTh, QTl, accs["q", cc]),
                                               (KTh, KTl, accs["k", cc])):
                            nc.any.tensor_copy(hi_t[:, cc, sl], ps[:])
                            nc.vector.tensor_tensor(lo_t[:, cc, sl], ps[:],
                                                    hi_t[:, cc, sl], SUB)
                    for r in range(KPB):
                        ki = qb * KPB + r
                        # strided dest: 4 groups of 64 V channels (skip ones col)
                        dst = Vsb[:, ki].rearrange("p (g w) -> p g w", w=VW)[:, :, 0:64]
                        nc.any.tensor_copy(dst, vacc[r][:])

            # ---------------- phase 2 + 3, software-pipelined ----------------
            with (
                tc.tile_pool(name="pssc", bufs=4, space="PSUM") as pssc,
                tc.tile_pool(name="psot", bufs=2, space="PSUM") as psot,
                tc.tile_pool(name="pso", bufs=1, space="PSUM") as pso,
                tc.tile_pool(name="stgp", bufs=22) as stgp,
                tc.tile_pool(name="expp", bufs=8) as expp,
            ):
                def phase3_block(j):
                    """out-proj for q block j (consumes that block's gather)."""
                    qsl3 = slice(j * QB, (j + 1) * QB)
                    accs = [pso.tile([P, QB], f32, tag=f"po{occ}",
                                     name=f"po{occ}")
                            for occ in range(NCC)]
                    last = (j == NQB - 1)
                    order = (sorted(range(DCH), key=lambda m: (m % NCC, m // NCC))
                             if last else list(range(DCH)))
                    for i, mch in enumerate(order):
                        g_, cc_ = mch // NCC, mch % NCC
                        mt = ms.tile([P, QB], bf16, tag="mt", name="mt")
                        if last:
                            nc.sync.dma_start(mt[:], ag_out_h[j, cc_][g_, :, :])
                        else:
                            nc.sync.dma_start(mt[:], ag_out[j][g_, :, cc_, :])
                        for occ in range(NCC):
                            for wi, wt in enumerate((woh, wol)):
                                nc.tensor.matmul(
                                    accs[occ][:], wt[:, mch, occ * P:(occ + 1) * P],
                                    mt[:], start=(i == 0 and wi == 0),
                                    stop=(i == DCH - 1 and wi == 1))
                    for occ in range(NCC):
                        oo = op.tile([P, QB], f32, tag="oo", name="oo")
                        nc.any.tensor_copy(oo[:], accs[occ][:])
                        nc.sync.dma_start(outT[occ, :, qsl3], oo[:])

                def _gather(inp, outp):
                    nc.gpsimd.collective_compute(
                        "AllGather", mybir.AluOpType.bypass,
                        replica_groups=[[0, 1, 2, 3], [4, 5, 6, 7]],
                        ins=[inp], outs=[outp],
                    )

                def passA(qb, hp, h2):
                    hsl = slice(h2 * 64, (h2 + 1) * 64)
                    nkc = qb * KPB + KPB
                    rm = stat.tile([P, QB], f32, tag="rm", name="rm")
                    nc.gpsimd.memset(rm[:], -3e38)
                    sts = []
                    for kc in range(nkc):
                        diag = kc - qb * KPB
                        off = max(0, diag) * P
                        psc = pssc.tile([P, QB], f32, tag="psc", name="psc")
                        ksl = slice(kc * P, (kc + 1) * P)
                        mvsl = slice(qb * QB + off, (qb + 1) * QB)
                        terms = ((KTh, QTh), (KTh, QTl), (KTl, QTh))
                        for ti, (kt, qt) in enumerate(terms):
                            nc.tensor.matmul(
                                psc[:, off:], kt[hsl, hp, ksl],
                                qt[hsl, hp, mvsl],
                                start=(ti == 0), stop=(ti == 2))
                        stg = stgp.tile([P, QB], f32, tag="stg", name="stg")
                        nc.scalar.activation(stg[:, off:], psc[:, off:], COPYF)
                        if diag >= 0:
                            nc.vector.tensor_tensor(
                                stg[:, off:off + P], stg[:, off:off + P],
                                maskT[:], mybir.AluOpType.add)
                        nc.vector.tensor_tensor(rm[:, off:], rm[:, off:],
                                                stg[:, off:], MAXOP)
                        sts.append((stg, off))
                    mrep = stat.tile([P, QB], f32, tag="mrep", name="mrep")
                    nc.gpsimd.partition_all_reduce(
                        mrep[:], rm[:], P, bass_isa.ReduceOp.max)
                    nc.vector.tensor_scalar_add(mrep[:], mrep[:], 6.0)
                    return (qb, hp, h2, sts, mrep)

                def passB(st):
                    qb, hp, h2, sts, mrep = st
                    qsl = slice(qb * QB, (qb + 1) * QB)
                    hsl = slice(h2 * 64, (h2 + 1) * 64)
                    nkc = len(sts)
                    otp = psot.tile([VW, QB], f32, tag="otp", name="otp")
                    vg = slice((hp * 2 + h2) * VW, (hp * 2 + h2 + 1) * VW)
                    for kc, (stg, off) in enumerate(sts):
                        nc.vector.tensor_tensor(stg[:, off:], stg[:, off:],
                                                mrep[:, off:], SUB)
                        ex = expp.tile([P, QB], bf16, tag="ex", name="ex")
                        nc.scalar.activation(ex[:, off:], stg[:, off:], EXP)
                        nc.tensor.matmul(otp[:, off:], Vsb[:, kc, vg],
                                         ex[:, off:],
                                         start=(kc == 0), stop=(kc == nkc - 1))
                    rec = stat.tile([1, QB], f32, tag="rec", name="rec")
                    nc.vector.reciprocal(rec[:], otp[64:65, :])
                    recb = stat.tile([64, QB], f32, tag="recb", name="recb")
                    nc.gpsimd.partition_broadcast(recb[:], rec[:], 64)
                    nc.vector.tensor_tensor(OT[hsl, hp, qsl], otp[0:64, :],
                                            recb[:], MULT)
                    # fire gathers / interleaved out-proj on block boundaries
                    if h2 == 1:
                        if qb == NQB - 1:
                            nc.sync.dma_start(ag_in_h[qb, hp][:],
                                              OT[:, hp, qsl])
                            _gather(ag_in_h[qb, hp][:], ag_out_h[qb, hp][:])
                        elif hp == NCC - 1:
                            nc.sync.dma_start(ag_in[qb][:], OT[:, :, qsl])
                            _gather(ag_in[qb][:], ag_out[qb][:])
                        if hp == NCC - 1 and qb >= 1:
                            phase3_block(qb - 1)

                blocks = [(qb, hp, h2) for qb in range(NQB)
                          for hp in range(NCC) for h2 in range(2)]
                prev = None
                for blk in blocks:
                    cur = passA(*blk)
                    if prev is not None:
                        passB(prev)
                    prev = cur
                passB(prev)
                phase3_block(NQB - 1)

    nc.compile()
    return nc


_NC_CACHE = {}


def get_nc(**cfg):
    key = tuple(sorted(cfg.items()))
    if key not in _NC_CACHE:
        _NC_CACHE[key] = build_nc(**cfg)
    return _NC_CACHE[key]


def _col_index(g):
    p = np.arange(CPC)
    return (p % HD) * HEADS + (HPC * g + p // HD)


def _ow_row_index():
    r = np.arange(D)
    m, p128 = r // P, r % P
    g_, cc = m // NCC, m % NCC
    p256 = cc * P + p128
    lh, hd = p256 // HD, p256 % HD
    return hd * HEADS + (HPC * g_ + lh)


def make_in_maps(x, qw, kw, vw, ow, s=S):
    scale = 1.0 / np.sqrt(np.float32(D))
    qws = (qw * scale).astype(np.float32)
    ow_perm = np.ascontiguousarray(ow[_ow_row_index()])
    in_maps = []
    xTs = [np.ascontiguousarray(x[b, :s].T) for b in range(B)]
    for c in range(N_CORES):
        b, g = c // GROUPS, c % GROUPS
        cols = _col_index(g)
        in_maps.append({
            "xT": xTs[b],
            "wq": np.ascontiguousarray(qws[:, cols]),
            "wk": np.ascontiguousarray(kw[:, cols]),
            "wv": np.ascontiguousarray(vw[:, cols]),
            "wo": np.ascontiguousarray(ow_perm[:, g * CPC:(g + 1) * CPC]),
        })
    return in_maps


def assemble_output(results, s=S):
    out = np.empty((B, s, D), dtype=np.float32)
    for c in range(N_CORES):
        b, g = c // GROUPS, c % GROUPS
        oT = results[c]["outT"]  # [NCC, P, s]
        for occ in range(NCC):
            out[b, :, g * CPC + occ * P:(g * CPC + (occ + 1) * P)] = oT[occ].T
    return out


def run_on_hw(x, qw, kw, vw, ow, trace=False, **cfg_over):
    cfg = dict(DEFAULT_CFG)
    cfg.update(cfg_over)
    s = cfg["s"]
    nc = get_nc(**cfg)
    in_maps = make_in_maps(x, qw, kw, vw, ow, s=s)
    res = run_bass_kernel_spmd(nc, in_maps, core_ids=list(range(N_CORES)),
                               trace=trace)
    return assemble_output(res.results, s=s), res


def kernel(x, qw, kw, vw, ow):
    out, _ = run_on_hw(np.asarray(x, dtype=np.float32),
                       np.asarray(qw, dtype=np.float32),
                       np.asarray(kw, dtype=np.float32),
                       np.asarray(vw, dtype=np.float32),
                       np.asarray(ow, dtype=np.float32))
    return out


# revision 17
# speedup vs baseline: 1.0332x; 1.0332x over previous
"""Multi-head attention (dense_transformer) on 8 TRN2 NeuronCores.

Sharding: 2-way data parallel over batch x 4-way tensor parallel over heads.
Core c handles batch b=c//4 and heads {4g..4g+3} where g=c%4 (4 heads, 256
channels per core; channels of head h are qw columns {hd*16+h}).

Architecture (v2, "transposed scores"):
  phase 1: Q^T/K^T projections in [ch, s] layout via 3-term bf16 splits
           (pseudo-fp32, needed because softmax here is a near-argmax: score
           std ~256, so score errors flip the max). V is projected DIRECTLY
           into [s, ch] layout with single-pass f32r matmuls (V error is
           linear in the output -> 11-bit mantissa is plenty), with a ones
           column appended per head so AV also produces the softmax sums.
  phase 2: scores computed TRANSPOSED: scT[k,q] = K^T-chunk (stationary) x
           Q^T (moving), 3-term bf16. Per 512-wide q block: drain chunks to
           SBUF, running column-max on gpsimd (Pool engine, otherwise idle),
           one partition_all_reduce(max) -> bias replicated on all
           partitions, DVE subtract + ACT exp -> expT bf16, then
           AV = Vhat-chunk (stationary [128,65]) x expT (moving) accumulates
           O^T[ch,q] AND l[q] in PSUM with no transposes at all.
           Normalize = reciprocal of l + gpsimd partition_broadcast + the
           PSUM drain is a tensor_tensor multiply.
  phase 3: AllGather O^T across the 4 cores of the batch -> out-proj column
           slice (out^T = ow_perm^T @ merged^T, f32r) -> DMA out.

vs v1: no PE transposes (was 576 matmuls / ~97us), V projection 3x cheaper,
no separate normalize pass, no exp accum; PE stream is denser so it holds
the 2.4GHz p-state instead of 1.2GHz.
"""
import sys

sys.path.insert(0, "/opt/trn_rl_repo")

import numpy as np

import concourse.bass as bass
import concourse.mybir as mybir
import concourse.tile as tile
from concourse import bacc
from concourse import bass_isa
from concourse.bass_utils import run_bass_kernel_spmd

# ---- problem constants (hardcoded per harness contract) ----
B, S, D, HEADS = 2, 2048, 1024, 16
N_CORES = 8
GROUPS = 4                 # head-groups == cores per batch
HPC = HEADS // GROUPS      # heads per core (4)
HD = D // HEADS            # 64
CPC = HPC * HD             # channels per core (256)
P = 128
NCC = CPC // P             # col chunks per core (2)
DCH = D // P               # contraction chunks (8)
QB = 512                   # q block width (1 PSUM bank of f32)

f32 = mybir.dt.float32
f32r = mybir.dt.float32r
bf16 = mybir.dt.bfloat16

AX = mybir.AxisListType
EXP = mybir.ActivationFunctionType.Exp
MAXOP = mybir.AluOpType.max
SUB = mybir.AluOpType.subtract
MULT = mybir.AluOpType.mult
COPYF = mybir.ActivationFunctionType.Copy

DEFAULT_CFG = dict(s=S)


def make_maskT(nc, maskT, mask_val=-1e10):
    """maskT[k, q] = 0 if q >= k else mask_val (transposed causal)."""
    sq = maskT.shape[0]
    nc.gpsimd.memset(maskT, mask_val)
    nc.gpsimd.affine_select(
        out=maskT,
        in_=maskT,
        compare_op=mybir.AluOpType.is_gt,
        fill=0.0,
        base=0,
        # keep mask_val where (k - q) > 0, else fill 0
        pattern=[[-1, sq]],
        channel_multiplier=1,
    )


def build_nc(s=S, dbg=False):
    assert s % QB == 0
    NQB = s // QB            # 512-wide q blocks
    NKC = s // P             # 128-wide k chunks
    KPB = QB // P            # k chunks per q block on the diagonal (4)
    VW = 65                  # V channels per (hp,h2) incl the ones column
    NH2 = NCC * 2            # head slots per core (4)

    nc = bacc.Bacc("TRN2", target_bir_lowering=False, debug=False,
                   num_devices=N_CORES)
    xT = nc.dram_tensor("xT", [D, s], f32, kind="ExternalInput").ap()
    wq = nc.dram_tensor("wq", [D, CPC], f32, kind="ExternalInput").ap()
    wk = nc.dram_tensor("wk", [D, CPC], f32, kind="ExternalInput").ap()
    wv = nc.dram_tensor("wv", [D, CPC], f32r, kind="ExternalInput").ap()
    wo = nc.dram_tensor("wo", [D, CPC], f32, kind="ExternalInput").ap()
    outT = nc.dram_tensor("outT", [NCC, P, s], f32, kind="ExternalOutput").ap()

    with tile.TileContext(nc) as tc:
        with (
            tc.tile_pool(name="cpool", bufs=1) as cpool,
            tc.tile_pool(name="wpool", bufs=1) as wpool,
            tc.tile_pool(name="big", bufs=1) as big,
            tc.tile_pool(name="stat", bufs=2) as stat,
            tc.tile_pool(name="ms", bufs=3) as ms,
            tc.tile_pool(name="op", bufs=2) as op,
            tc.tile_pool(name="dram", bufs=1, space="DRAM") as dpool,
        ):
            NQB_ = s // QB
            ag_in = {qb: dpool.tile([P, NCC, QB], bf16, tag=f"agi{qb}",
                                    name=f"agi{qb}")
                     for qb in range(NQB_ - 1)}
            ag_out = {qb: dpool.tile([GROUPS, P, NCC, QB], bf16,
                                     tag=f"ago{qb}", name=f"ago{qb}")
                      for qb in range(NQB_ - 1)}
            ag_in_h = {(NQB_ - 1, hp): dpool.tile([P, QB], bf16,
                                                  tag=f"agih{hp}",
                                                  name=f"agih{hp}")
                       for hp in range(NCC)}
            ag_out_h = {(NQB_ - 1, hp): dpool.tile([GROUPS, P, QB], bf16,
                                                   tag=f"agoh{hp}",
                                                   name=f"agoh{hp}")
                        for hp in range(NCC)}

            maskT = cpool.tile([P, P], f32, tag="maskT")
            make_maskT(nc, maskT[:])

            woh = wpool.tile([P, DCH, CPC], bf16, tag="woh")
            wol = wpool.tile([P, DCH, CPC], bf16, tag="wol")
            wv_sb = wpool.tile([P, DCH, CPC], f32r, tag="wv")
            for di in range(DCH):
                nc.sync.dma_start(wv_sb[:, di, :], wv[di * P:(di + 1) * P, :])
            wsplit = {}
            for nm in ("q", "k"):
                wh = wpool.tile([P, DCH, CPC], bf16, tag=f"w{nm}h", name=f"w{nm}h")
                wl = wpool.tile([P, DCH, CPC], bf16, tag=f"w{nm}l", name=f"w{nm}l")
                wsplit[nm] = [wh, wl]
            with tc.tile_pool(name="wload", bufs=2) as wload:
                for nm, wdr in (("q", wq), ("k", wk), ("o", wo)):
                    wf = wload.tile([P, DCH, CPC], f32, tag="wf", name="wf")
                    for di in range(DCH):
                        nc.sync.dma_start(wf[:, di, :],
                                          wdr[di * P:(di + 1) * P, :])
                    wh, wl = ((woh, wol) if nm == "o" else wsplit[nm])
                    nc.vector.tensor_copy(wh[:], wf[:])
                    nc.vector.tensor_tensor(wl[:], wf[:], wh[:], SUB)

            QTh = big.tile([P, NCC, s], bf16, tag="QTh")
            QTl = big.tile([P, NCC, s], bf16, tag="QTl")
            KTh = big.tile([P, NCC, s], bf16, tag="KTh")
            KTl = big.tile([P, NCC, s], bf16, tag="KTl")
            # Vhat[k, :]: 4 groups of 65 cols: 64 V channels + a ones col
            Vsb = big.tile([P, NKC, NH2 * VW], bf16, tag="Vsb")
            OT = big.tile([P, NCC, s], bf16, tag="OT")
            stage = big.tile([P, NKC, QB], f32, tag="stage")
            expT = big.tile([P, NKC, QB], bf16, tag="expT")

            for g in range(NH2):
                nc.gpsimd.memset(Vsb[:, :, g * VW + 64], 1.0)

            # ---------------- phase 1: projections ----------------
            with (
                tc.tile_pool(name="psp", bufs=1, space="PSUM") as psp,
                tc.tile_pool(name="psv", bufs=1, space="PSUM") as psv,
                tc.tile_pool(name="xs", bufs=5) as xs,
            ):
                for qb in range(NQB):
                    accs = {}
                    for nm in ("q", "k"):
                        for cc in range(NCC):
                            accs[nm, cc] = psp.tile([P, QB], f32,
                                                    tag=f"pp{nm}{cc}",
                                                    name=f"pp{nm}{cc}")
                    vacc = [psv.tile([P, CPC], f32, tag=f"pv{r}", name=f"pv{r}")
                            for r in range(KPB)]
                    for di in range(DCH):
                        xt = xs.tile([P, QB], f32, tag="xt", name="xt")
                        nc.sync.dma_start(
                            xt[:], xT[di * P:(di + 1) * P, qb * QB:(qb + 1) * QB])
                        xth = xs.tile([P, QB], bf16, tag="xth", name="xth")
                        xtl = xs.tile([P, QB], bf16, tag="xtl", name="xtl")
                        nc.vector.tensor_copy(xth[:], xt[:])
                        nc.vector.tensor_tensor(xtl[:], xt[:], xth[:], SUB)
                        xtr = xs.tile([P, QB], f32r, tag="xtr", name="xtr")
                        nc.any.tensor_copy(xtr[:], xt[:])
                        for nm in ("q", "k"):
                            wh, wl = wsplit[nm]
                            for cc in range(NCC):
                                csl = slice(cc * P, (cc + 1) * P)
                                terms = [(wh, xth), (wh, xtl), (wl, xth)]
                                for ti, (wt, xtt) in enumerate(terms):
                                    nc.tensor.matmul(
                                        accs[nm, cc][:], wt[:, di, csl], xtt[:],
                                        start=(di == 0 and ti == 0),
                                        stop=(di == DCH - 1 and ti == len(terms) - 1))
                        for r in range(KPB):
                            nc.tensor.matmul(
                                vacc[r][:], xtr[:, r * P:(r + 1) * P],
                                wv_sb[:, di, :],
                                start=(di == 0), stop=(di == DCH - 1))
                    sl = slice(qb * QB, (qb + 1) * QB)
                    for cc in range(NCC):
                        for hi_t, lo_t, ps in ((QTh, QTl, accs["q", cc]),
                                               (KTh, KTl, accs["k", cc])):
                            nc.any.tensor_copy(hi_t[:, cc, sl], ps[:])
                            nc.vector.tensor_tensor(lo_t[:, cc, sl], ps[:],
                                                    hi_t[:, cc, sl], SUB)
                    for r in range(KPB):
                        ki = qb * KPB + r
                        # strided dest: 4 groups of 64 V channels (skip ones col)
                        dst = Vsb[:, ki].rearrange("p (g w) -> p g w", w=VW)[:, :, 0:64]
                        nc.any.tensor_copy(dst, vacc[r][:])

            # ---------------- phase 2 + 3, software-pipelined ----------------
            with (
                tc.tile_pool(name="pssc", bufs=4, space="PSUM") as pssc,
                tc.tile_pool(name="psot", bufs=2, space="PSUM") as psot,
                tc.tile_pool(name="pso", bufs=1, space="PSUM") as pso,
                tc.tile_pool(name="stgp", bufs=22) as stgp,
                tc.tile_pool(name="expp", bufs=8) as expp,
            ):
                def phase3_block(j):
                    """out-proj for q block j (consumes that block's gather)."""
                    qsl3 = slice(j * QB, (j + 1) * QB)
                    accs = [pso.tile([P, QB], f32, tag=f"po{occ}",
                                     name=f"po{occ}")
                            for occ in range(NCC)]
                    last = (j == NQB - 1)
                    order = (sorted(range(DCH), key=lambda m: (m % NCC, m // NCC))
                             if last else list(range(DCH)))
                    for i, mch in enumerate(order):
                        g_, cc_ = mch // NCC, mch % NCC
                        mt = ms.tile([P, QB], bf16, tag="mt", name="mt")
                        if last:
                            nc.sync.dma_start(mt[:], ag_out_h[j, cc_][g_, :, :])
                        else:
                            nc.sync.dma_start(mt[:], ag_out[j][g_, :, cc_, :])
                        for occ in range(NCC):
                            for wi, wt in enumerate((woh, wol)):
                                nc.tensor.matmul(
                                    accs[occ][:], wt[:, mch, occ * P:(occ + 1) * P],
                                    mt[:], start=(i == 0 and wi == 0),
                                    stop=(i == DCH - 1 and wi == 1))
                    for occ in range(NCC):
                        oo = op.tile([P, QB], f32, tag="oo", name="oo")
                        nc.any.tensor_copy(oo[:], accs[occ][:])
                        nc.sync.dma_start(outT[occ, :, qsl3], oo[:])

                def _gather(inp, outp):
                    nc.gpsimd.collective_compute(
                        "AllGather", mybir.AluOpType.bypass,
                        replica_groups=[[0, 1, 2, 3], [4, 5, 6, 7]],
                        ins=[inp], outs=[outp],
                    )

                def passA(qb, hp, h2):
                    hsl = slice(h2 * 64, (h2 + 1) * 64)
                    nkc = qb * KPB + KPB
                    rm = stat.tile([P, QB], f32, tag="rm", name="rm")
                    nc.gpsimd.memset(rm[:], -3e38)
                    sts = []
                    for kc in range(nkc):
                        diag = kc - qb * KPB
                        off = max(0, diag) * P
                        psc = pssc.tile([P, QB], f32, tag="psc", name="psc")
                        ksl = slice(kc * P, (kc + 1) * P)
                        mvsl = slice(qb * QB + off, (qb + 1) * QB)
                        terms = ((KTh, QTh), (KTh, QTl), (KTl, QTh))
                        for ti, (kt, qt) in enumerate(terms):
                            nc.tensor.matmul(
                                psc[:, off:], kt[hsl, hp, ksl],
                                qt[hsl, hp, mvsl],
                                start=(ti == 0), stop=(ti == 2))
                        stg = stgp.tile([P, QB], f32, tag="stg", name="stg")
                        nc.scalar.activation(stg[:, off:], psc[:, off:], COPYF)
                        if diag >= 0:
                            nc.vector.tensor_tensor(
                                stg[:, off:off + P], stg[:, off:off + P],
                                maskT[:], mybir.AluOpType.add)
                        nc.vector.tensor_tensor(rm[:, off:], rm[:, off:],
                                                stg[:, off:], MAXOP)
                        sts.append((stg, off))
                    mrep = stat.tile([P, QB], f32, tag="mrep", name="mrep")
                    nc.gpsimd.partition_all_reduce(
                        mrep[:], rm[:], P, bass_isa.ReduceOp.max)
                    nc.vector.tensor_scalar_add(mrep[:], mrep[:], 6.0)
                    return (qb, hp, h2, sts, mrep)

                def passB(st):
                    qb, hp, h2, sts, mrep = st
                    qsl = slice(qb * QB, (qb + 1) * QB)
                    hsl = slice(h2 * 64, (h2 + 1) * 64)
                    nkc = len(sts)
                    otp = psot.tile([VW, QB], f32, tag="otp", name="otp")
                    vg = slice((hp * 2 + h2) * VW, (hp * 2 + h2 + 1) * VW)
                    for kc, (stg, off) in enumerate(sts):
                        nc.vector.tensor_tensor(stg[:, off:], stg[:, off:],
                                                mrep[:, off:], SUB)
                        ex = expp.tile([P, QB], bf16, tag="ex", name="ex")
                        nc.scalar.activation(ex[:, off:], stg[:, off:], EXP)
                        nc.tensor.matmul(otp[:, off:], Vsb[:, kc, vg],
                                         ex[:, off:],
                                         start=(kc == 0), stop=(kc == nkc - 1))
                    rec = stat.tile([1, QB], f32, tag="rec", name="rec")
                    nc.vector.reciprocal(rec[:], otp[64:65, :])
                    recb = stat.tile([64, QB], f32, tag="recb", name="recb")
                    nc.gpsimd.partition_broadcast(recb[:], rec[:], 64)
                    nc.vector.tensor_tensor(OT[hsl, hp, qsl], otp[0:64, :],
                                            recb[:], MULT)
                    # fire gathers / interleaved out-proj on block boundaries
                    if h2 == 1:
                        if qb == NQB - 1:
                            nc.sync.dma_start(ag_in_h[qb, hp][:],
                                              OT[:, hp, qsl])
                            _gather(ag_in_h[qb, hp][:], ag_out_h[qb, hp][:])
                        elif hp == NCC - 1:
                            nc.sync.dma_start(ag_in[qb][:], OT[:, :, qsl])
                            _gather(ag_in[qb][:], ag_out[qb][:])
                        if hp == NCC - 1 and qb >= 2:
                            phase3_block(qb - 2)

                blocks = [(qb, hp, h2) for qb in range(NQB)
                          for hp in range(NCC) for h2 in range(2)]
                prev = None
                for blk in blocks:
                    cur = passA(*blk)
                    if prev is not None:
                        passB(prev)
                    prev = cur
                passB(prev)
                for j in range(max(0, NQB - 2), NQB):
                    phase3_block(j)

    nc.compile()
    return nc


_NC_CACHE = {}


def get_nc(**cfg):
    key = tuple(sorted(cfg.items()))
    if key not in _NC_CACHE:
        _NC_CACHE[key] = build_nc(**cfg)
    return _NC_CACHE[key]


def _col_index(g):
    p = np.arange(CPC)
    return (p % HD) * HEADS + (HPC * g + p // HD)


def _ow_row_index():
    r = np.arange(D)
    m, p128 = r // P, r % P
    g_, cc = m // NCC, m % NCC
    p256 = cc * P + p128
    lh, hd = p256 // HD, p256 % HD
    return hd * HEADS + (HPC * g_ + lh)


def make_in_maps(x, qw, kw, vw, ow, s=S):
    scale = 1.0 / np.sqrt(np.float32(D))
    qws = (qw * scale).astype(np.float32)
    ow_perm = np.ascontiguousarray(ow[_ow_row_index()])
    in_maps = []
    xTs = [np.ascontiguousarray(x[b, :s].T) for b in range(B)]
    for c in range(N_CORES):
        b, g = c // GROUPS, c % GROUPS
        cols = _col_index(g)
        in_maps.append({
            "xT": xTs[b],
            "wq": np.ascontiguousarray(qws[:, cols]),
            "wk": np.ascontiguousarray(kw[:, cols]),
            "wv": np.ascontiguousarray(vw[:, cols]),
            "wo": np.ascontiguousarray(ow_perm[:, g * CPC:(g + 1) * CPC]),
        })
    return in_maps


def assemble_output(results, s=S):
    out = np.empty((B, s, D), dtype=np.float32)
    for c in range(N_CORES):
        b, g = c // GROUPS, c % GROUPS
        oT = results[c]["outT"]  # [NCC, P, s]
        for occ in range(NCC):
            out[b, :, g * CPC + occ * P:(g * CPC + (occ + 1) * P)] = oT[occ].T
    return out


def run_on_hw(x, qw, kw, vw, ow, trace=False, **cfg_over):
    cfg = dict(DEFAULT_CFG)
    cfg.update(cfg_over)
    s = cfg["s"]
    nc = get_nc(**cfg)
    in_maps = make_in_maps(x, qw, kw, vw, ow, s=s)
    res = run_bass_kernel_spmd(nc, in_maps, core_ids=list(range(N_CORES)),
                               trace=trace)
    return assemble_output(res.results, s=s), res


def kernel(x, qw, kw, vw, ow):
    out, _ = run_on_hw(np.asarray(x, dtype=np.float32),
                       np.asarray(qw, dtype=np.float32),
                       np.asarray(kw, dtype=np.float32),
                       np.asarray(vw, dtype=np.float32),
                       np.asarray(ow, dtype=np.float32))
    return out
